# revision 19
# baseline (speedup 1.0000x reference)
"""FLC pooling (FFT2 -> center-crop low freqs -> IFFT2, real part) on 8 trn2 cores.

Math: per (n,c) slice, out = Re(M @ X @ M.T) where M (112x224) is the 1D
fft -> fftshift -> crop -> ifftshift -> ifft operator. Im(M) is exactly
rank-1 (= outer(a, b), a[u] = a0*(-1)^u), so with R = Re(M), G = [R; b]:

    out_ext = G @ X @ G.T            (113x113; [112,112] entry = b'Xb)
    out = out_ext[:112,:112] - out_ext[112,112] * a0^2 * checkerboard

Device pipeline (fp16 operands, fp32 PSUM accumulation):
    W1T = X.T @ G.T      pass 1: stationary = X chunks (fp16), streams G.T;
                         produces the *transposed* intermediate directly,
                         so no PE transposes / identity are needed.
    V   = G @ W1T        pass 2: = out_ext^T, k slices batched (N=113k),
                         fp16 x fp16 -> fp32 PSUM
    s   = b.T X b        tiny matmul against W1T col 112, broadcast to
                         all partitions via a constant-column lhsT
    vout = cneg*s + V    one fused DVE scalar_tensor_tensor per slice
Host unshard transposes (free re-layout).

x is loaded by gpsimd casting DMA (fp32 HBM -> fp16 SBUF); each
partition reads one contiguous 1792B run (two adjacent rows) per slice.
All DVE results accumulate in one [112, 128, 112] fp16 SBUF tile and
the output (fp16, upcast on host) is written by TWO giant DMAs at the
END into a [v, slice, u] HBM layout (26.9KB contiguous per partition).
Deferring the output is the key scheduling decision: the input stream
then gets exclusive HBM bandwidth (~410GB/s read vs ~320 when output
DMAs interleave), so the last input bytes - and the compute chains
hanging off them - finish ~8us earlier, and the output streams at
~360GB/s while the trailing 2-slice groups (slices 120-127, individual
small DMAs, short dependency chains) finish underneath it. The first
output chunk [0:120] becomes eligible right as the input drains; any
output issued earlier steals stream bandwidth and cascades (measured
+6..10us). Tensor is NOT the limiter (a 4-slice group takes ~1.06us:
pass-1 matmuls retire every ~52ns); the kernel is data-arrival-bound
end to end, modulo +-4us of run-to-run p-state/DVFS throttle noise.

Sharding: batch*channel = 1024 independent (n,c) slices -> 128 per core.
"""

import sys

sys.path.insert(0, "/opt/trn_rl_repo")

import numpy as np

import concourse.bass as bass  # noqa: F401
import concourse.mybir as mybir
import concourse.tile as tile
from concourse import bacc
from concourse.bass_utils import run_bass_kernel_spmd

N = 224
NH = 112
NG = 113  # rows of G = [R; b]
B, C = 16, 64
NCORES = 8
NSLICES = B * C // NCORES  # 128 slices per core
NBIG = 30  # 4-slice groups (slices 0..119)
NTAIL = 4  # trailing 2-slice groups (slices 120..127)
F32 = mybir.dt.float32
F16 = mybir.dt.float16


def _build_consts():
    F = np.fft.fft(np.eye(N), axis=0, norm="forward")
    M = np.fft.ifft(
        np.fft.ifftshift(np.fft.fftshift(F, axes=0)[N // 4 : 3 * N // 4], axes=0),
        axis=0,
        norm="forward",
    )
    R, S = M.real, M.imag
    u, sv, vt = np.linalg.svd(S)
    a = u[:, 0] * np.sqrt(sv[0])
    b = vt[0] * np.sqrt(sv[0])
    if np.abs(S - np.outer(a, b)).max() > 1e-10:
        a, b = -a, -b
    assert np.abs(S - np.outer(a, b)).max() < 1e-12
    G = np.vstack([R, b[None, :]])  # [113, 224]
    # gt16[c][i, u] = G[u, 112c + i]  (G^T row chunks, fp16; pass-2 lhsT)
    gt16 = np.ascontiguousarray(G.T.reshape(2, NH, NG)).astype(np.float16)
    # gtp16[e][p, u] = G[u, 2p + e]  (G^T rows by parity, fp16; pass-1 rhs --
    # pairs with x loaded two-adjacent-rows-per-partition)
    gtp16 = np.ascontiguousarray(
        G.T.reshape(NH, 2, NG).transpose(1, 0, 2)
    ).astype(np.float16)
    # bbc16[c][j, m] = b[112c + j] for all m (column-broadcast b)
    bbc16 = np.ascontiguousarray(
        np.repeat(b.reshape(2, NH, 1), NH, axis=2)
    ).astype(np.float16)
    a0sq = float(a[0] * a[0])  # = 1/224
    vv = np.arange(NH)
    cneg = (-a0sq * ((-1.0) ** (vv[:, None] + vv[None, :]))).astype(np.float32)
    return gt16, gtp16, bbc16, cneg


def _build_nc():
    nc = bacc.Bacc("TRN2", target_bir_lowering=False, debug=False)
    x = nc.dram_tensor("x", [NSLICES, N, N], F32, kind="ExternalInput").ap()
    gt = nc.dram_tensor("gt", [2, NH, NG], F16, kind="ExternalInput").ap()
    gtp = nc.dram_tensor("gtp", [2, NH, NG], F16, kind="ExternalInput").ap()
    bbc = nc.dram_tensor("bbc", [2, NH, NH], F16, kind="ExternalInput").ap()
    cneg = nc.dram_tensor("cneg", [NH, NH], F32, kind="ExternalInput").ap()
    # [v, slice, u]: per-group writes are 896B-contiguous per partition
    outT = nc.dram_tensor("outT", [NH, NSLICES, NH], F16, kind="ExternalOutput").ap()

    mult = mybir.AluOpType.mult
    add = mybir.AluOpType.add

    with tile.TileContext(nc) as tc:
        with (
            tc.tile_pool(name="consts", bufs=1) as cpool,
            tc.tile_pool(name="xt", bufs=8) as xpool,
            tc.tile_pool(name="xt2", bufs=4) as x2pool,
            tc.tile_pool(name="w1t4", bufs=6) as w1t4_pool,
            tc.tile_pool(name="vout", bufs=1) as vout_pool,
            tc.tile_pool(name="w1tp", bufs=3, space="PSUM") as w1tpsum,
            tc.tile_pool(name="v4p", bufs=2, space="PSUM") as vpsum,
            tc.tile_pool(name="s4p", bufs=2, space="PSUM") as spsum,
        ):
            gt_sb = cpool.tile([NH, 2, NG], F16)
            nc.sync.dma_start(gt_sb[:], gt.rearrange("c i u -> i c u"))
            gtp_sb = cpool.tile([NH, 2, NG], F16)
            nc.sync.dma_start(gtp_sb[:], gtp.rearrange("e p u -> p e u"))
            bbc_sb = cpool.tile([NH, 2, NH], F16)
            nc.sync.dma_start(bbc_sb[:], bbc.rearrange("c j m -> j c m"))
            cneg_sb = cpool.tile([NH, NH], F32)
            nc.sync.dma_start(cneg_sb[:], cneg)

            # All DVE results land here; outputs are written by two giant
            # contiguous DMAs at the END so the input stream never shares
            # HBM bandwidth with output traffic (which would delay the
            # last input bytes and serialize the final compute chains).
            vout_all = vout_pool.tile([NH, NSLICES, NH], F16)

            def group(s0, k, xt):
                """Compute k slices starting at s0 from xt [NH, k, 2N]."""
                # w1tk[p, h, s, u] = W1T_s[112h + p, u] = W1_s[u, 112h + p]
                w1tk = w1t4_pool.tile([NH, 2, k, NG], F16, tag="w1t")
                for q in range(k // 2):  # slice pairs
                    w1tp = w1tpsum.tile([NH, 2, 2, NG], F32, tag="w1tp")
                    for si in range(2):
                        sl = 2 * q + si
                        for h in range(2):  # W1T row chunk (j)
                            for e in range(2):  # contraction chunk (i parity)
                                nc.tensor.matmul(
                                    w1tp[:, si, h, :],
                                    xt[:, sl, e * N + h * NH : e * N + (h + 1) * NH],
                                    gtp_sb[:, e, :],
                                    start=(e == 0),
                                    stop=(e == 1),
                                )
                    nc.scalar.copy(
                        w1tk[:, :, 2 * q : 2 * q + 2, :],
                        w1tp[:].rearrange("p si h u -> p h si u"),
                    )
                vk = vpsum.tile([NG, k, NG], F32, tag="v")
                sk = spsum.tile([NH, k], F32, tag="s")
                for h in range(2):
                    nc.tensor.matmul(
                        vk[:],
                        gt_sb[:, h, :],
                        w1tk[:, h],
                        start=(h == 0),
                        stop=(h == 1),
                    )
                for h in range(2):
                    nc.tensor.matmul(
                        sk[:],
                        bbc_sb[:, h, :],
                        w1tk[:, h, :, NH : NH + 1],
                        start=(h == 0),
                        stop=(h == 1),
                    )
                for sl in range(k):
                    # vout = cneg * s + V  (fused correction + PSUM eviction)
                    nc.vector.scalar_tensor_tensor(
                        out=vout_all[:, s0 + sl, :],
                        in0=cneg_sb[:],
                        scalar=sk[:, sl : sl + 1],
                        in1=vk[0:NH, sl, 0:NH],
                        op0=mult,
                        op1=add,
                    )

            # 30 4-slice groups; 8-slice casting DMAs (one per tile pair)
            for g in range(NBIG):
                if g % 2 == 0:
                    xt8 = xpool.tile([NH, 8, 2 * N], F16, tag="xt")
                    nc.gpsimd.dma_start(
                        xt8[:],
                        x[4 * g : 4 * g + 8].rearrange(
                            "s (p e) j -> p s (e j)", e=2
                        ),
                    )
                group(4 * g, 4, xt8[:, 4 * (g % 2) : 4 * (g % 2) + 4, :])

            # trailing 2-slice groups with individual small DMAs: the
            # dependency chain after the last input byte is one short
            # 2-slice compute instead of two serialized 4-slice chains
            for t in range(NTAIL):
                s0 = 4 * NBIG + 2 * t
                xt2 = x2pool.tile([NH, 2, 2 * N], F16, tag="xt2")
                nc.gpsimd.dma_start(
                    xt2[:],
                    x[s0 : s0 + 2].rearrange("s (p e) j -> p s (e j)", e=2),
                )
                group(s0, 2, xt2[:])

            # Output writeout: the big chunk on the Sync ring, gated on
            # the first tail group's DVEs (#~122) — which fire one short
            # 2-slice chain after the last input byte, so it can never
            # steal stream bandwidth. The tiny last chunk goes on the
            # (empty by then) gpsimd ring so the two chunks overlap
            # instead of serializing on one ring. (Putting any output
            # with an earlier dep on either ring was measured to jump
            # ahead of / interleave with pending input descriptors.)
            nc.sync.dma_start(outT[:, 0:120, :], vout_all[:, 0:120, :])
            nc.gpsimd.dma_start(
                outT[:, 120:NSLICES, :], vout_all[:, 120:NSLICES, :]
            )
    nc.compile()
    return nc


_CACHE: dict = {}


def _get_compiled():
    if "nc" not in _CACHE:
        _CACHE["consts"] = _build_consts()
        _CACHE["nc"] = _build_nc()
    return _CACHE["nc"], _CACHE["consts"]


def run(x: np.ndarray, trace: bool = False):
    """Returns (out [16,64,112,112] fp32, BassKernelResults)."""
    nc, (gt16, gtp16, bbc16, cneg) = _get_compiled()
    x = np.ascontiguousarray(np.asarray(x, dtype=np.float32))
    shards = x.reshape(NCORES, NSLICES, N, N)
    in_maps = [
        {"x": shards[i], "gt": gt16, "gtp": gtp16, "bbc": bbc16, "cneg": cneg}
        for i in range(NCORES)
    ]
    last_err = None
    for _attempt in range(3):
        try:
            res = run_bass_kernel_spmd(
                nc, in_maps, core_ids=list(range(NCORES)), trace=trace
            )
            break
        except Exception as e:  # transient NRT device errors: retry
            last_err = e
    else:
        raise last_err
    outT = np.stack([r["outT"] for r in res.results], axis=0)  # [8, v, s, u]
    out = np.ascontiguousarray(
        outT.astype(np.float32).transpose(0, 2, 3, 1)  # [8, s, u, v]
    ).reshape(B, C, NH, NH)
    return out, res


def kernel(x: np.ndarray) -> np.ndarray:
    out, _ = run(x, trace=False)
    return out


# revision 20
# speedup vs baseline: 1.0070x; 1.0070x over previous
"""FLC pooling (FFT2 -> center-crop low freqs -> IFFT2, real part) on 8 trn2 cores.

Math: per (n,c) slice, out = Re(M @ X @ M.T) where M (112x224) is the 1D
fft -> fftshift -> crop -> ifftshift -> ifft operator. Im(M) is exactly
rank-1 (= outer(a, b), a[u] = a0*(-1)^u), so with R = Re(M), G = [R; b]:

    out_ext = G @ X @ G.T            (113x113; [112,112] entry = b'Xb)
    out = out_ext[:112,:112] - out_ext[112,112] * a0^2 * checkerboard

Device pipeline (fp16 operands, fp32 PSUM accumulation):
    W1T = X.T @ G.T      pass 1: stationary = X chunks (fp16), streams G.T;
                         produces the *transposed* intermediate directly,
                         so no PE transposes / identity are needed.
    V   = G @ W1T        pass 2: = out_ext^T, k slices batched (N=113k),
                         fp16 x fp16 -> fp32 PSUM
    s   = b.T X b        tiny matmul against W1T col 112, broadcast to
                         all partitions via a constant-column lhsT
    vout = cneg*s + V    one fused DVE scalar_tensor_tensor per slice
Host unshard transposes (free re-layout).

x is loaded by gpsimd casting DMA (fp32 HBM -> fp16 SBUF); each
partition reads one contiguous 1792B run (two adjacent rows) per slice.
All DVE results accumulate in one [112, 128, 112] fp16 SBUF tile and
the output (fp16, upcast on host) is written by TWO giant DMAs at the
END into a [v, slice, u] HBM layout (26.9KB contiguous per partition).
Deferring the output is the key scheduling decision: the input stream
then gets exclusive HBM bandwidth (~410GB/s read vs ~320 when output
DMAs interleave), so the last input bytes - and the compute chains
hanging off them - finish ~8us earlier, and the output streams at
~360GB/s while the trailing 2-slice groups (slices 120-127, individual
small DMAs, short dependency chains) finish underneath it. The first
output chunk [0:120] becomes eligible right as the input drains; any
output issued earlier steals stream bandwidth and cascades (measured
+6..10us). Tensor is NOT the limiter (a 4-slice group takes ~1.06us:
pass-1 matmuls retire every ~52ns); the kernel is data-arrival-bound
end to end, modulo +-4us of run-to-run p-state/DVFS throttle noise.

Sharding: batch*channel = 1024 independent (n,c) slices -> 128 per core.
"""

import sys

sys.path.insert(0, "/opt/trn_rl_repo")

import numpy as np

import concourse.bass as bass  # noqa: F401
import concourse.mybir as mybir
import concourse.tile as tile
from concourse import bacc
from concourse.bass_utils import run_bass_kernel_spmd

N = 224
NH = 112
NG = 113  # rows of G = [R; b]
B, C = 16, 64
NCORES = 8
NSLICES = B * C // NCORES  # 128 slices per core
NBIG = 30  # 4-slice groups (slices 0..119)
NTAIL = 4  # trailing 2-slice groups (slices 120..127)
F32 = mybir.dt.float32
F16 = mybir.dt.float16


def _build_consts():
    F = np.fft.fft(np.eye(N), axis=0, norm="forward")
    M = np.fft.ifft(
        np.fft.ifftshift(np.fft.fftshift(F, axes=0)[N // 4 : 3 * N // 4], axes=0),
        axis=0,
        norm="forward",
    )
    R, S = M.real, M.imag
    u, sv, vt = np.linalg.svd(S)
    a = u[:, 0] * np.sqrt(sv[0])
    b = vt[0] * np.sqrt(sv[0])
    if np.abs(S - np.outer(a, b)).max() > 1e-10:
        a, b = -a, -b
    assert np.abs(S - np.outer(a, b)).max() < 1e-12
    G = np.vstack([R, b[None, :]])  # [113, 224]
    # gt16[c][i, u] = G[u, 112c + i]  (G^T row chunks, fp16; pass-2 lhsT)
    gt16 = np.ascontiguousarray(G.T.reshape(2, NH, NG)).astype(np.float16)
    # gtp16[e][p, u] = G[u, 2p + e]  (G^T rows by parity, fp16; pass-1 rhs --
    # pairs with x loaded two-adjacent-rows-per-partition)
    gtp16 = np.ascontiguousarray(
        G.T.reshape(NH, 2, NG).transpose(1, 0, 2)
    ).astype(np.float16)
    # bbc16[c][j, m] = b[112c + j] for all m (column-broadcast b)
    bbc16 = np.ascontiguousarray(
        np.repeat(b.reshape(2, NH, 1), NH, axis=2)
    ).astype(np.float16)
    a0sq = float(a[0] * a[0])  # = 1/224
    vv = np.arange(NH)
    cneg = (-a0sq * ((-1.0) ** (vv[:, None] + vv[None, :]))).astype(np.float32)
    return gt16, gtp16, bbc16, cneg


def _build_nc():
    nc = bacc.Bacc("TRN2", target_bir_lowering=False, debug=False)
    x = nc.dram_tensor("x", [NSLICES, N, N], F32, kind="ExternalInput").ap()
    gt = nc.dram_tensor("gt", [2, NH, NG], F16, kind="ExternalInput").ap()
    gtp = nc.dram_tensor("gtp", [2, NH, NG], F16, kind="ExternalInput").ap()
    bbc = nc.dram_tensor("bbc", [2, NH, NH], F16, kind="ExternalInput").ap()
    cneg = nc.dram_tensor("cneg", [NH, NH], F32, kind="ExternalInput").ap()
    # [v, slice, u]: per-group writes are 896B-contiguous per partition
    outT = nc.dram_tensor("outT", [NH, NSLICES, NH], F16, kind="ExternalOutput").ap()

    mult = mybir.AluOpType.mult
    add = mybir.AluOpType.add

    with tile.TileContext(nc) as tc:
        with (
            tc.tile_pool(name="consts", bufs=1) as cpool,
            tc.tile_pool(name="xt", bufs=8) as xpool,
            tc.tile_pool(name="xt2", bufs=4) as x2pool,
            tc.tile_pool(name="w1t4", bufs=6) as w1t4_pool,
            tc.tile_pool(name="vout", bufs=1) as vout_pool,
            tc.tile_pool(name="w1tp", bufs=3, space="PSUM") as w1tpsum,
            tc.tile_pool(name="v4p", bufs=2, space="PSUM") as vpsum,
            tc.tile_pool(name="s4p", bufs=2, space="PSUM") as spsum,
        ):
            gt_sb = cpool.tile([NH, 2, NG], F16)
            nc.sync.dma_start(gt_sb[:], gt.rearrange("c i u -> i c u"))
            gtp_sb = cpool.tile([NH, 2, NG], F16)
            nc.sync.dma_start(gtp_sb[:], gtp.rearrange("e p u -> p e u"))
            bbc_sb = cpool.tile([NH, 2, NH], F16)
            nc.sync.dma_start(bbc_sb[:], bbc.rearrange("c j m -> j c m"))
            cneg_sb = cpool.tile([NH, NH], F32)
            nc.sync.dma_start(cneg_sb[:], cneg)

            # All DVE results land here; outputs are written by two giant
            # contiguous DMAs at the END so the input stream never shares
            # HBM bandwidth with output traffic (which would delay the
            # last input bytes and serialize the final compute chains).
            vout_all = vout_pool.tile([NH, NSLICES, NH], F16)

            def group(s0, k, xt):
                """Compute k slices starting at s0 from xt [NH, k, 2N]."""
                # w1tk[p, h, s, u] = W1T_s[112h + p, u] = W1_s[u, 112h + p]
                w1tk = w1t4_pool.tile([NH, 2, k, NG], F16, tag="w1t")
                for q in range(k // 2):  # slice pairs
                    w1tp = w1tpsum.tile([NH, 2, 2, NG], F32, tag="w1tp")
                    for si in range(2):
                        sl = 2 * q + si
                        for h in range(2):  # W1T row chunk (j)
                            for e in range(2):  # contraction chunk (i parity)
                                nc.tensor.matmul(
                                    w1tp[:, si, h, :],
                                    xt[:, sl, e * N + h * NH : e * N + (h + 1) * NH],
                                    gtp_sb[:, e, :],
                                    start=(e == 0),
                                    stop=(e == 1),
                                )
                    # PSUM->SBUF eviction split by h so each V matmul can
                    # start after half a copy instead of a full one
                    for h in range(2):
                        nc.scalar.copy(
                            w1tk[:, h, 2 * q : 2 * q + 2, :],
                            w1tp[:, :, h, :],
                        )
                vk = vpsum.tile([NG, k, NG], F32, tag="v")
                sk = spsum.tile([NH, k], F32, tag="s")
                for h in range(2):
                    nc.tensor.matmul(
                        vk[:],
                        gt_sb[:, h, :],
                        w1tk[:, h],
                        start=(h == 0),
                        stop=(h == 1),
                    )
                for h in range(2):
                    nc.tensor.matmul(
                        sk[:],
                        bbc_sb[:, h, :],
                        w1tk[:, h, :, NH : NH + 1],
                        start=(h == 0),
                        stop=(h == 1),
                    )
                for sl in range(k):
                    # vout = cneg * s + V  (fused correction + PSUM eviction)
                    nc.vector.scalar_tensor_tensor(
                        out=vout_all[:, s0 + sl, :],
                        in0=cneg_sb[:],
                        scalar=sk[:, sl : sl + 1],
                        in1=vk[0:NH, sl, 0:NH],
                        op0=mult,
                        op1=add,
                    )

            # 30 4-slice groups; 8-slice casting DMAs (one per tile pair)
            for g in range(NBIG):
                if g % 2 == 0:
                    xt8 = xpool.tile([NH, 8, 2 * N], F16, tag="xt")
                    nc.gpsimd.dma_start(
                        xt8[:],
                        x[4 * g : 4 * g + 8].rearrange(
                            "s (p e) j -> p s (e j)", e=2
                        ),
                    )
                group(4 * g, 4, xt8[:, 4 * (g % 2) : 4 * (g % 2) + 4, :])

            # trailing 2-slice groups with individual small DMAs: the
            # dependency chain after the last input byte is one short
            # 2-slice compute instead of two serialized 4-slice chains
            for t in range(NTAIL):
                s0 = 4 * NBIG + 2 * t
                xt2 = x2pool.tile([NH, 2, 2 * N], F16, tag="xt2")
                nc.gpsimd.dma_start(
                    xt2[:],
                    x[s0 : s0 + 2].rearrange("s (p e) j -> p s (e j)", e=2),
                )
                group(s0, 2, xt2[:])

            # Output writeout: the big chunk on the Sync ring, gated on
            # the first tail group's DVEs (#~122) — which fire one short
            # 2-slice chain after the last input byte, so it can never
            # steal stream bandwidth. The tiny last chunk goes on the
            # (empty by then) gpsimd ring so the two chunks overlap
            # instead of serializing on one ring. (Putting any output
            # with an earlier dep on either ring was measured to jump
            # ahead of / interleave with pending input descriptors.)
            nc.sync.dma_start(outT[:, 0:120, :], vout_all[:, 0:120, :])
            nc.gpsimd.dma_start(
                outT[:, 120:NSLICES, :], vout_all[:, 120:NSLICES, :]
            )
    nc.compile()
    return nc


_CACHE: dict = {}


def _get_compiled():
    if "nc" not in _CACHE:
        _CACHE["consts"] = _build_consts()
        _CACHE["nc"] = _build_nc()
    return _CACHE["nc"], _CACHE["consts"]


def run(x: np.ndarray, trace: bool = False):
    """Returns (out [16,64,112,112] fp32, BassKernelResults)."""
    nc, (gt16, gtp16, bbc16, cneg) = _get_compiled()
    x = np.ascontiguousarray(np.asarray(x, dtype=np.float32))
    shards = x.reshape(NCORES, NSLICES, N, N)
    in_maps = [
        {"x": shards[i], "gt": gt16, "gtp": gtp16, "bbc": bbc16, "cneg": cneg}
        for i in range(NCORES)
    ]
    last_err = None
    for _attempt in range(3):
        try:
            res = run_bass_kernel_spmd(
                nc, in_maps, core_ids=list(range(NCORES)), trace=trace
            )
            break
        except Exception as e:  # transient NRT device errors: retry
            last_err = e
    else:
        raise last_err
    outT = np.stack([r["outT"] for r in res.results], axis=0)  # [8, v, s, u]
    out = np.ascontiguousarray(
        outT.astype(np.float32).transpose(0, 2, 3, 1)  # [8, s, u, v]
    ).reshape(B, C, NH, NH)
    return out, res


def kernel(x: np.ndarray) -> np.ndarray:
    out, _ = run(x, trace=False)
    return out
